# revision 1
# baseline (speedup 1.0000x reference)
"""Multi-head attention (B=2, S=2048, D=1024, H=16) on 8 Trainium2 cores.

Sharding: batch x head-block. Core c handles batch b=c//4 and 4 heads
starting at h0=4*(c%4). Per core:
  1. QKV projections in transposed layout (fp32r matmuls, full rate):
     qw^T/kw^T = W_slice^T-free via lhsT=W (natural), rhs=x^T (host-transposed);
     vw natural via lhsT=v^T blocks, rhs=Wv. Biases fused (DVE per-partition
     scalar add for qw^T/kw^T; K=1 ones-matmul for vw, which also writes the
     ones columns used to fuse softmax-denominator sums into the PV matmul).
  2. Attention per head: scores^T [k,q] with K=64 matmuls packed two-per-array
     via row strips (head A at partitions 0-63, head B at 64-127); exp on ACT
     (scale=1/8 fused, no max subtraction -- scores are N(0,1)); PV+sums in one
     matmul stream via [vw | ones] lhsT; normalize with reciprocal_approx_fast.
  3. Two 8-core AllToAlls (one per head pair) exchange ctx^T so each core
     gets all 1024 channels for its 512-row query slice; pair 0's exchange
     hides under pair 1's compute. Chunks are double-sent to both batch
     groups' block positions so the program stays SPMD-static; the receiving
     side multiplies the other batch's half by host-zeroed Wo rows.
  4. Output projection in two passes (pair-0 channels while pair-1's
     exchange is in flight) + bias, direct disjoint slice out.
Host assembles the 8 disjoint [512,1024] slices.
"""
import contextlib
import ctypes
import os
import sys
import types

import ml_dtypes
import numpy as np

for _p in ("/opt/trn_rl_repo", os.path.expanduser("~/.axon_site/_ro/trn_rl_repo")):
    if os.path.isdir(_p) and _p not in sys.path:
        sys.path.insert(0, _p)
        break


def _install_ntff_hook():
    """run_bass_kernel_spmd(trace=True) under axon imports antenv.axon_hooks,
    which this image lacks; provide it so tracing degrades gracefully."""
    if "antenv.axon_hooks" in sys.modules:
        return
    mod = types.ModuleType("antenv.axon_hooks")
    state = {"hook": None}
    mod.set_axon_ntff_profile_hook = lambda h: state.__setitem__("hook", h)
    mod.get_axon_ntff_profile_hook = lambda: state["hook"]
    sys.modules["antenv.axon_hooks"] = mod
    try:
        import antenv

        antenv.axon_hooks = mod
    except ImportError:
        pass
    so_path = "/opt/axon/libaxon_pjrt.so"
    try:
        lib = ctypes.CDLL(so_path)
        if not hasattr(lib, "axon_start_nrt_profile"):
            return
        lib.axon_start_nrt_profile.argtypes = [
            ctypes.POINTER(ctypes.c_int64), ctypes.c_size_t]
        lib.axon_start_nrt_profile.restype = ctypes.c_int64
        lib.axon_stop_nrt_profile.argtypes = [ctypes.c_char_p]
        lib.axon_stop_nrt_profile.restype = ctypes.c_int64

        @contextlib.contextmanager
        def _ctx(output_dir, device_ids):
            import jax

            jax.devices()
            if device_ids:
                ids = (ctypes.c_int64 * len(device_ids))(*device_ids)
                rc = lib.axon_start_nrt_profile(ids, len(device_ids))
            else:
                rc = lib.axon_start_nrt_profile(None, 0)
            if rc != 0:
                raise RuntimeError(f"axon_start_nrt_profile rc={rc}")
            try:
                yield
            finally:
                n = lib.axon_stop_nrt_profile(str(output_dir).encode())
                print(f"profile: {n} ntff file(s) in {output_dir}",
                      file=sys.stderr)

        state["hook"] = _ctx
    except OSError:
        pass


_install_ntff_hook()

import concourse.bacc as bacc  # noqa: E402
import concourse.mybir as mybir  # noqa: E402
import concourse.tile as tile  # noqa: E402
from concourse.bass_utils import run_bass_kernel_spmd  # noqa: E402

F32 = mybir.dt.float32
F32R = mybir.dt.float32r
BF16 = mybir.dt.bfloat16
AF = mybir.ActivationFunctionType
MUL = mybir.AluOpType.mult

N_CORES = 8
B, S, D, H, HD = 2, 2048, 1024, 16, 64
HPC = 4            # heads per core
DPC = HPC * HD     # 256 output dims per core
NCH = 4            # q chunks of 512
QW = S // NCH      # 512
KT = S // 128      # 16 k-position tiles
DKT = D // 128     # 8 d_model contraction tiles

_CACHED_NC = None


def _build():
    nc = bacc.Bacc("TRN2", target_bir_lowering=False, debug=False,
                   num_devices=N_CORES)

    # per-core inputs (SPMD program; data differs per core), arranged as
    # [chunk, partition, k-tile, col] so chunk DMAs read 8KB-contiguous runs
    qT = nc.dram_tensor("qT", [NCH, 128, DKT, QW], BF16,
                        kind="ExternalInput").ap()
    kT = nc.dram_tensor("kT", [NCH, 128, DKT, QW], BF16,
                        kind="ExternalInput").ap()
    vT = nc.dram_tensor("vT", [NCH, 128, DKT, QW], BF16,
                        kind="ExternalInput").ap()
    # weights pre-arranged host-side as [128, DKT, n] (contiguous per
    # partition -> large DMA descriptors)
    wq = nc.dram_tensor("wq", [128, DKT, DPC], BF16,
                        kind="ExternalInput").ap()
    wk = nc.dram_tensor("wk", [128, DKT, DPC], BF16,
                        kind="ExternalInput").ap()
    wv = nc.dram_tensor("wv", [128, DKT, DPC], BF16,
                        kind="ExternalInput").ap()
    bq2 = nc.dram_tensor("bq2", [128, 2], F32, kind="ExternalInput").ap()
    bk2 = nc.dram_tensor("bk2", [128, 2], F32, kind="ExternalInput").ap()
    bvx = nc.dram_tensor("bvx", [1, 512], BF16, kind="ExternalInput").ap()
    wo2 = nc.dram_tensor("wo2", [2, 128, DKT, D], BF16,
                         kind="ExternalInput").ap()
    bo1 = nc.dram_tensor("bo1", [1, D], BF16, kind="ExternalInput").ap()
    out = nc.dram_tensor("out", [QW, D], F32, kind="ExternalOutput").ap()

    taps = {}
    if os.environ.get("DEBUG_TAPS"):
        taps["tqwT"] = nc.dram_tensor("tqwT", [128, 2, S], F32R,
                                      kind="ExternalOutput").ap()
        taps["tkwT"] = nc.dram_tensor("tkwT", [128, 2, S], F32R,
                                      kind="ExternalOutput").ap()

    with tile.TileContext(nc) as tc:
        with tc.tile_pool(name="xw", bufs=1) as xw, \
             tc.tile_pool(name="dram", bufs=1, space="DRAM") as dram:
            # long-lived projection outputs
            qwT = xw.tile([128, 2, S], F32R, name="qwT")   # pair-major d_out
            kwT = xw.tile([128, 2, S], F32R, name="kwT")
            vwx = xw.tile([128, KT, 512], BF16, name="vwx")  # [vw64|ones64] x4
            onesr = xw.tile([1, 128], F32R, name="onesr")
            bq_sb = xw.tile([128, 2], F32, name="bq_sb")
            bk_sb = xw.tile([128, 2], F32, name="bk_sb")
            bvx_sb = xw.tile([1, 512], BF16, name="bvx_sb")
            onesb = xw.tile([1, 128], BF16, name="onesb")
            bo_sb = xw.tile([1, D], BF16, name="bo_sb")

            ones_f = xw.tile([1, 128], F32, name="ones_f")
            nc.gpsimd.memset(ones_f[:], 1.0)
            nc.vector.tensor_copy(onesr[:], ones_f[:])
            nc.vector.tensor_copy(onesb[:], ones_f[:])
            nc.sync.dma_start(out=bq_sb[:], in_=bq2[:])
            nc.sync.dma_start(out=bk_sb[:], in_=bk2[:])
            nc.sync.dma_start(out=bvx_sb[:], in_=bvx[:])
            nc.sync.dma_start(out=bo_sb[:], in_=bo1[:])

            cin0 = dram.tile([2 * 512, QW], BF16, name="cin0")
            cout0 = dram.tile([2 * 512, QW], BF16, name="cout0")
            cin1 = dram.tile([2 * 512, QW], BF16, name="cin1")
            cout1 = dram.tile([2 * 512, QW], BF16, name="cout1")
            cins, couts = (cin0, cin1), (cout0, cout1)
            # full-size scratch warmup exchange (garbage data) absorbs the
            # NRT first-collective setup cost off the critical path
            ccw_in = dram.tile([2 * 512, QW], BF16, name="ccw_in")
            ccw_out = dram.tile([2 * 512, QW], BF16, name="ccw_out")
            # pass-1 half of the (permuted) output weights, prefetched early
            wo_a = xw.tile([128, DKT, D], BF16, name="wo_a")

            # ---- phase 1: projections ----
            with tc.tile_pool(name="wpool", bufs=1) as wp, \
                 tc.tile_pool(name="xt", bufs=4) as xtp, \
                 tc.tile_pool(name="pps", bufs=2, space="PSUM") as pps:
                wq_sb = wp.tile([128, DKT, DPC], BF16, name="wq_sb")
                wk_sb = wp.tile([128, DKT, DPC], BF16, name="wk_sb")
                wv_sb = wp.tile([128, DKT, DPC], BF16, name="wv_sb")

                def xchunk(x_dram, ch, name):
                    t = xtp.tile([128, DKT, QW], BF16, name=name, tag="xt")
                    h = DKT // 2
                    nc.sync.dma_start(out=t[:, 0:h, :],
                                      in_=x_dram[ch][:, 0:h, :])
                    nc.scalar.dma_start(out=t[:, h:DKT, :],
                                        in_=x_dram[ch][:, h:DKT, :])
                    return t

                # first v-chunk ahead of the weights: the very first matmul
                # needs vt0 (1MB) + wv (0.5MB) -- don't queue 1.5MB of other
                # weights in front of it
                vt0 = xchunk(vT, 0, "vt")
                nc.sync.dma_start(out=wv_sb[:], in_=wv[:])
                nc.scalar.dma_start(out=wk_sb[:], in_=wk[:])
                nc.sync.dma_start(out=wq_sb[:], in_=wq[:])
                # wo pass-1 half + collective warmup, hidden on gpsimd
                nc.gpsimd.dma_start(out=wo_a[:], in_=wo2[0])
                nc.gpsimd.collective_compute(
                    "AllToAll", mybir.AluOpType.bypass,
                    replica_groups=[list(range(N_CORES))],
                    ins=[ccw_in[:].opt()], outs=[ccw_out[:].opt()])

                # vw (+bias, +ones cols): vwx[:, sblk] = [4x(vw64|ones64)]
                for ch in range(NCH):
                    vt = vt0 if ch == 0 else xchunk(vT, ch, "vt")
                    for sb_i in range(4):
                        sblk = ch * 4 + sb_i
                        ps = pps.tile([128, 512], F32, name="psv", tag="ps")
                        for kk in range(DKT):
                            nc.tensor.matmul(
                                ps[:, 0:DPC],
                                vt[:, kk, sb_i * 128:(sb_i + 1) * 128],
                                wv_sb[:, kk, :],
                                start=(kk == 0), stop=False)
                        # K=1 ones-matmul: adds bv to cols 0:256, writes 1.0
                        # into cols 256:512 (ones for the fused sums)
                        nc.tensor.matmul(ps[:], onesb[:], bvx_sb[:],
                                         start=False, stop=True)
                        dst = vwx[:, sblk, :].rearrange(
                            "p (h c) -> p h c", h=HPC)
                        nc.vector.tensor_copy(
                            dst[:, :, 64:128],
                            ps[:, 0:DPC].rearrange("p (h c) -> p h c", h=HPC))
                        # ones FIRST in each head block: PV then lands the
                        # softmax sums at PSUM base 0, where the custom DVE
                        # reciprocal can read them directly (no shift copy)
                        nc.vector.tensor_copy(
                            dst[:, :, 0:64],
                            ps[:, DPC:512].rearrange("p (h c) -> p h c", h=HPC))

                # kw^T then qw^T: [128,2,S], rows = pair-major d_out
                for x_dram, w_sb, b_sb, dstT in (
                        (kT, wk_sb, bk_sb, kwT), (qT, wq_sb, bq_sb, qwT)):
                    for ch in range(NCH):
                        xt = xchunk(x_dram, ch, "xt")
                        for m in range(2):
                            ps = pps.tile([128, QW], F32, name="ps", tag="ps")
                            for kk in range(DKT):
                                nc.tensor.matmul(
                                    ps[:],
                                    w_sb[:, kk, m * 128:(m + 1) * 128],
                                    xt[:, kk, :],
                                    start=(kk == 0), stop=(kk == DKT - 1))
                            nc.vector.tensor_scalar_add(
                                dstT[:, m, ch * QW:(ch + 1) * QW],
                                ps[:], b_sb[:, m:m + 1])

            # ---- phase 2: attention (units software-pipelined so ACT
            # never idles at unit boundaries; per-pair A2A so pair 0's
            # exchange hides under pair 1's compute) ----
            with tc.tile_pool(name="probs", bufs=40) as prp, \
                 tc.tile_pool(name="stg", bufs=4) as stp, \
                 tc.tile_pool(name="sps", bufs=3, space="PSUM") as sps, \
                 tc.tile_pool(name="vps", bufs=2, space="PSUM") as vps:

                def emit_scores(pair, ch, kts):
                    prs = []
                    for kt in kts:
                        sq = sps.tile([128, 2, 512], F32, name="sq", tag="sq")
                        for dh in range(2):
                            nc.tensor.matmul(
                                sq[:, dh, :],
                                kwT[dh * 64:(dh + 1) * 64, pair,
                                    kt * 128:(kt + 1) * 128],
                                qwT[dh * 64:(dh + 1) * 64, pair,
                                    ch * QW:(ch + 1) * QW],
                                start=True, stop=True)
                        pr = prp.tile([128, 2, 512], BF16, name="pr", tag="pr")
                        nc.scalar.activation(pr[:], sq[:], AF.Exp, scale=0.125)
                        prs.append(pr)
                    return prs

                def emit_pvs(pair, ch, prs, dhs=(0, 1)):
                    for dh in dhs:
                        lh = 2 * pair + dh
                        # fused PV+sums: lhsT=[vw|ones] -> ctx rows 0:64,
                        # sums rows 64:128
                        pv = vps.tile([128, 512], F32, name="pv", tag="pv")
                        for kt in range(KT):
                            nc.tensor.matmul(
                                pv[:],
                                vwx[:, kt, lh * 128:(lh + 1) * 128],
                                prs[kt][:, dh, :],
                                start=(kt == 0), stop=(kt == KT - 1))
                        # sums now sit at PSUM rows 0:64 (base 0), so the
                        # custom DVE reciprocal reads them directly
                        rec = stp.tile([64, 512], F32, name="rec", tag="rec")
                        nc.vector.reciprocal_approx_fast(rec[:], pv[0:64, :])
                        stg = stp.tile([64, 512], BF16, name="stg", tag="stg")
                        nc.vector.tensor_tensor(stg[:], pv[64:128, :], rec[:],
                                                MUL)
                        # double-send: both batch groups' block positions
                        row = ch * 128 + dh * 64
                        nc.sync.dma_start(
                            out=cins[pair][row:row + 64, :], in_=stg[:])
                        nc.sync.dma_start(
                            out=cins[pair][512 + row:512 + row + 64, :],
                            in_=stg[:])

                def emit_a2a(pair):
                    nc.gpsimd.collective_compute(
                        "AllToAll", mybir.AluOpType.bypass,
                        replica_groups=[list(range(N_CORES))],
                        ins=[cins[pair][:].opt()],
                        outs=[couts[pair][:].opt()])

                # half-size PV blocks (one head per slot: dh0 right after its
                # own chunk's scores, dh1 one chunk later) keep ACT streaming
                # through the (pair, chunk) boundaries
                pend = None
                for pair in range(2):
                    for ch in range(NCH):
                        prs = emit_scores(pair, ch, range(KT // 2))
                        if pend is not None:
                            emit_pvs(*pend, dhs=(1,))
                            if pend[0] == 0 and pend[1] == NCH - 1:
                                emit_a2a(0)
                        prs += emit_scores(pair, ch, range(KT // 2, KT))
                        emit_pvs(pair, ch, prs, dhs=(0,))
                        pend = (pair, ch, prs)
                emit_pvs(*pend, dhs=(1,))
                emit_a2a(1)

            # ---- phase 3: output projection ----
            if taps:
                nc.sync.dma_start(out=taps["tqwT"][:], in_=qwT[:])
                nc.sync.dma_start(out=taps["tkwT"][:], in_=kwT[:])

            with tc.tile_pool(name="op", bufs=1) as op, \
                 tc.tile_pool(name="osb", bufs=2) as osb, \
                 tc.tile_pool(name="ops", bufs=8, space="PSUM") as ops:
                # keep the PE clock warm across the collective wait
                warm = ops.tile([128, 512], F32, name="warm", tag="pso")
                for i in range(16):
                    nc.tensor.matmul(warm[:], onesb[:], bo_sb[:, 0:512],
                                     start=(i == 0), stop=(i == 15))
                wo_b = op.tile([128, DKT, D], BF16, name="wo_b")
                nc.scalar.dma_start(out=wo_b[:], in_=wo2[1])
                gth0 = op.tile([128, DKT, QW], BF16, name="gth0")
                gth1 = op.tile([128, DKT, QW], BF16, name="gth1")
                for gth, cout in ((gth0, cout0), (gth1, cout1)):
                    src = cout.rearrange("(k p) n -> p k n", p=128)
                    nc.sync.dma_start(out=gth[:, 0:DKT // 2, :],
                                      in_=src[:, 0:DKT // 2, :])
                    nc.scalar.dma_start(out=gth[:, DKT // 2:DKT, :],
                                        in_=src[:, DKT // 2:DKT, :])

                pss = {}
                # pass 1: pair-0 channels (runs while pair-1 A2A is in flight)
                for mb in range(QW // 128):
                    for nch in range(2):
                        ps = ops.tile([128, 512], F32, name="pso", tag="pso")
                        pss[(mb, nch)] = ps
                        for kk in range(DKT):
                            nc.tensor.matmul(
                                ps[:],
                                gth0[:, kk, mb * 128:(mb + 1) * 128],
                                wo_a[:, kk, nch * 512:(nch + 1) * 512],
                                start=(kk == 0), stop=False)
                # biases need no gathered data: add them during the A2A wait
                for mb in range(QW // 128):
                    for nch in range(2):
                        nc.tensor.matmul(
                            pss[(mb, nch)][:], onesb[:],
                            bo_sb[:, nch * 512:(nch + 1) * 512],
                            start=False, stop=False)
                # bridge the A2A-1 wait so pass 2 starts at full clock
                for i in range(32):
                    nc.tensor.matmul(warm[:], onesb[:], bo_sb[:, 0:512],
                                     start=(i == 0), stop=(i == 31))
                # pass 2: pair-1 channels, then copy out
                for mb in range(QW // 128):
                    osb_t = osb.tile([128, D], F32, name="osb_t", tag="osb")
                    for nch in range(2):
                        ps = pss[(mb, nch)]
                        for kk in range(DKT):
                            nc.tensor.matmul(
                                ps[:],
                                gth1[:, kk, mb * 128:(mb + 1) * 128],
                                wo_b[:, kk, nch * 512:(nch + 1) * 512],
                                start=False, stop=(kk == DKT - 1))
                        nc.vector.tensor_copy(
                            osb_t[:, nch * 512:(nch + 1) * 512], ps[:])
                    nc.sync.dma_start(
                        out=out[mb * 128:(mb + 1) * 128, :], in_=osb_t[:])

    nc.compile()
    return nc


def _get_nc():
    global _CACHED_NC
    if _CACHED_NC is None:
        _CACHED_NC = _build()
    return _CACHED_NC


def kernel(q, k, v, Wq, bq, Wk, bk, Wv, bv, Wo, bo, _return_results=False):
    q, k, v = (np.asarray(x, np.float32) for x in (q, k, v))
    Wq, bq, Wk, bk, Wv, bv, Wo, bo = (
        np.asarray(x, np.float32) for x in (Wq, bq, Wk, bk, Wv, bv, Wo, bo))

    nc = _get_nc()
    in_maps = []
    for c in range(N_CORES):
        b, j = c // 4, c % 4
        cols = slice(4 * j * HD, 4 * j * HD + DPC)
        wo2 = np.zeros((2, 8, 128, D), np.float32)
        for p in range(2):
            for r in range(4 * b, 4 * b + 4):
                base = 256 * (r % 4) + 128 * p
                wo2[p, r] = Wo[base:base + 128]
        # [plane, partition, k-tile, n] with 16KB-contiguous runs
        wo2 = np.ascontiguousarray(
            wo2.transpose(0, 2, 1, 3)).astype(ml_dtypes.bfloat16)

        def arrange(x):
            xA = x.T.reshape(DKT, 128, NCH, QW).transpose(2, 1, 0, 3)
            return np.ascontiguousarray(xA).astype(ml_dtypes.bfloat16)

        def warrange(w):  # [D, n] -> [128, DKT, n]
            n = w.shape[1]
            return np.ascontiguousarray(
                w.reshape(DKT, 128, n).transpose(1, 0, 2)).astype(
                ml_dtypes.bfloat16)

        in_maps.append({
            "qT": arrange(q[b]),
            "kT": arrange(k[b]),
            "vT": arrange(v[b]),
            "wq": warrange(Wq[:, cols]),
            "wk": warrange(Wk[:, cols]),
            "wv": warrange(Wv[:, cols]),
            "bq2": np.ascontiguousarray(bq[cols].reshape(2, 128).T),
            "bk2": np.ascontiguousarray(bk[cols].reshape(2, 128).T),
            "bvx": np.concatenate([bv[cols], np.ones(DPC, np.float32)]).reshape(1, 512).astype(ml_dtypes.bfloat16),
            "wo2": wo2,
            "bo1": bo.reshape(1, D).astype(ml_dtypes.bfloat16),
        })

    res = run_bass_kernel_spmd(nc, in_maps, core_ids=list(range(N_CORES)))

    full = np.empty((B, S, D), np.float32)
    for c in range(N_CORES):
        b, j = c // 4, c % 4
        full[b, j * QW:(j + 1) * QW] = res.results[c]["out"]
    if _return_results:
        return full, res
    return full



# revision 8
# speedup vs baseline: 1.0870x; 1.0870x over previous
"""Multi-head attention (B=2, S=2048, D=1024, H=16) on 8 Trainium2 cores.

Sharding: head x batch. Core c handles heads {2c, 2c+1} for BOTH batches
(instead of 4 heads x 1 batch). This makes the post-attention exchange a
clean 8-way AllToAll per batch with zero redundancy: core c sends its
[128ch, 512q] blocks and receives exactly its [1024ch, 256q] output slice
(core c owns queries [256c, 256c+256) of each batch). Compared to the
batch-split sharding this halves the A2A payload (2 x 512KB, batch-0's
exchange hidden under batch-1 attention) and halves the output projection
(contract 1024 real channels, no zero-padded half).

Per core:
  1. Projections, interleaved with attention so the PE never idles:
     qw^T/kw^T in transposed layout (bf16 -- full-rate scores matmuls and
     half-size LDWEIGHTS vs fp32r), bias fused via DVE per-partition add;
     vw first as vw^T (N=512 matmuls, per-partition DVE bias), then PE
     transposes [128,128] blocks into the natural [kpos, ch] layout the
     PV matmul needs, with static memset ones columns for the fused
     softmax-denominator sums.
  2. Attention per (batch, q-chunk): scores^T [k,q] with K=64 matmuls
     packed two-per-array via row strips (head A partitions 0-63, head B
     64-127); exp on ACT (scale=1/8, no max subtraction -- scores are
     N(0,1)); PV+sums in one matmul stream via [ones|vw] lhsT; normalize
     with reciprocal_approx_fast. ACT is saturated here; projections for
     the other batch and the first batch's output projection fill the
     tensor-engine slack.
  3. Two 8-way AllToAlls (one per batch). Batch 0's fires mid-kernel and
     hides under batch-1 attention; only batch 1's is tail-exposed.
  4. Output projection per batch: bias first (start=True ones-matmul,
     PSUM-resident during the A2A wait), then 8 contraction steps over
     the gathered [1024ch, 256q] slice.
Host assembles the 8 disjoint [2, 256, 1024] slices.
"""
import contextlib
import ctypes
import os
import sys
import types

import ml_dtypes
import numpy as np

for _p in ("/opt/trn_rl_repo", os.path.expanduser("~/.axon_site/_ro/trn_rl_repo")):
    if os.path.isdir(_p) and _p not in sys.path:
        sys.path.insert(0, _p)
        break


def _install_ntff_hook():
    """run_bass_kernel_spmd(trace=True) under axon imports antenv.axon_hooks,
    which this image lacks; provide it so tracing degrades gracefully."""
    if "antenv.axon_hooks" in sys.modules:
        return
    mod = types.ModuleType("antenv.axon_hooks")
    state = {"hook": None}
    mod.set_axon_ntff_profile_hook = lambda h: state.__setitem__("hook", h)
    mod.get_axon_ntff_profile_hook = lambda: state["hook"]
    sys.modules["antenv.axon_hooks"] = mod
    try:
        import antenv

        antenv.axon_hooks = mod
    except ImportError:
        pass
    so_path = "/opt/axon/libaxon_pjrt.so"
    try:
        lib = ctypes.CDLL(so_path)
        if not hasattr(lib, "axon_start_nrt_profile"):
            return
        lib.axon_start_nrt_profile.argtypes = [
            ctypes.POINTER(ctypes.c_int64), ctypes.c_size_t]
        lib.axon_start_nrt_profile.restype = ctypes.c_int64
        lib.axon_stop_nrt_profile.argtypes = [ctypes.c_char_p]
        lib.axon_stop_nrt_profile.restype = ctypes.c_int64

        @contextlib.contextmanager
        def _ctx(output_dir, device_ids):
            import jax

            jax.devices()
            if device_ids:
                ids = (ctypes.c_int64 * len(device_ids))(*device_ids)
                rc = lib.axon_start_nrt_profile(ids, len(device_ids))
            else:
                rc = lib.axon_start_nrt_profile(None, 0)
            if rc != 0:
                raise RuntimeError(f"axon_start_nrt_profile rc={rc}")
            try:
                yield
            finally:
                n = lib.axon_stop_nrt_profile(str(output_dir).encode())
                print(f"profile: {n} ntff file(s) in {output_dir}",
                      file=sys.stderr)

        state["hook"] = _ctx
    except OSError:
        pass


_install_ntff_hook()

import concourse.bacc as bacc  # noqa: E402
import concourse.mybir as mybir  # noqa: E402
import concourse.tile as tile  # noqa: E402
from concourse.bass_utils import run_bass_kernel_spmd  # noqa: E402

F32 = mybir.dt.float32
BF16 = mybir.dt.bfloat16
AF = mybir.ActivationFunctionType
MUL = mybir.AluOpType.mult

N_CORES = 8
B, S, D, H, HD = 2, 2048, 1024, 16, 64
DPC = 2 * HD       # 128 projection columns per core (2 heads)
NCH = 4            # q chunks of 512 per batch
QW = S // NCH      # 512
QO = 256           # output queries per (core, batch)
KT = S // 128      # 16 k-position tiles per batch
DKT = D // 128     # 8 d_model contraction tiles

_CACHED_NC = None


def _build():
    nc = bacc.Bacc("TRN2", target_bir_lowering=False, debug=False,
                   num_devices=N_CORES)

    # x tensors hold BOTH batches: chunk index cidx = b*4 + ch, arranged as
    # [cidx, partition(d_in%128), k-tile(d_in//128), seq] so chunk DMAs read
    # 8KB-contiguous runs per partition
    qT = nc.dram_tensor("qT", [2 * NCH, 128, DKT, QW], BF16,
                        kind="ExternalInput").ap()
    kT = nc.dram_tensor("kT", [2 * NCH, 128, DKT, QW], BF16,
                        kind="ExternalInput").ap()
    vT = nc.dram_tensor("vT", [2 * NCH, 128, DKT, QW], BF16,
                        kind="ExternalInput").ap()
    wq = nc.dram_tensor("wq", [128, DKT, DPC], BF16,
                        kind="ExternalInput").ap()
    wk = nc.dram_tensor("wk", [128, DKT, DPC], BF16,
                        kind="ExternalInput").ap()
    wv = nc.dram_tensor("wv", [128, DKT, DPC], BF16,
                        kind="ExternalInput").ap()
    bq1 = nc.dram_tensor("bq1", [128, 1], F32, kind="ExternalInput").ap()
    bk1 = nc.dram_tensor("bk1", [128, 1], F32, kind="ExternalInput").ap()
    bv1 = nc.dram_tensor("bv1", [128, 1], F32, kind="ExternalInput").ap()
    # Wo rows permuted to the gathered-channel order: row (j*128 + dh*64 + d)
    # holds Wo[(2j+dh)*64 + d, :]
    wo1 = nc.dram_tensor("wo1", [128, DKT, D], BF16,
                         kind="ExternalInput").ap()
    bo1 = nc.dram_tensor("bo1", [1, D], BF16, kind="ExternalInput").ap()
    ident = nc.dram_tensor("ident", [128, 128], BF16,
                           kind="ExternalInput").ap()
    out = nc.dram_tensor("out", [2, QO, D], F32, kind="ExternalOutput").ap()

    taps = {}
    if os.environ.get("DEBUG_TAPS"):
        taps["tqwT"] = nc.dram_tensor("tqwT", [128, 2, S], BF16,
                                      kind="ExternalOutput").ap()
        taps["tkwT"] = nc.dram_tensor("tkwT", [128, 2, S], BF16,
                                      kind="ExternalOutput").ap()
        taps["tvwx"] = nc.dram_tensor("tvwx", [128, 2, KT, 256], BF16,
                                      kind="ExternalOutput").ap()

    with tile.TileContext(nc) as tc:
        with tc.tile_pool(name="xw", bufs=1) as xw, \
             tc.tile_pool(name="dram", bufs=1, space="DRAM") as dram:
            # long-lived tiles
            qwT = xw.tile([128, 2, S], BF16, name="qwT")   # [dh*64+d, b, q]
            kwT = xw.tile([128, 2, S], BF16, name="kwT")
            # [kpos%128, b, kt, (ones64|vw64) x2 dh]
            vwx = xw.tile([128, 2, KT, 256], BF16, name="vwx")
            bq_sb = xw.tile([128, 1], F32, name="bq_sb")
            bk_sb = xw.tile([128, 1], F32, name="bk_sb")
            bv_sb = xw.tile([128, 1], F32, name="bv_sb")
            onesb = xw.tile([1, 128], BF16, name="onesb")
            bo_sb = xw.tile([1, D], BF16, name="bo_sb")
            id_sb = xw.tile([128, 128], BF16, name="id_sb")
            wo_sb = xw.tile([128, DKT, D], BF16, name="wo_sb")
            gth0 = xw.tile([128, DKT, QO], BF16, name="gth0")
            gth1 = xw.tile([128, DKT, QO], BF16, name="gth1")
            wq_sb = xw.tile([128, DKT, DPC], BF16, name="wq_sb")
            wk_sb = xw.tile([128, DKT, DPC], BF16, name="wk_sb")
            wv_sb = xw.tile([128, DKT, DPC], BF16, name="wv_sb")

            ones_f = xw.tile([1, 128], F32, name="ones_f")
            nc.gpsimd.memset(ones_f[:], 1.0)
            nc.vector.tensor_copy(onesb[:], ones_f[:])
            # static ones columns of vwx (softmax-denominator lhsT rows);
            # two 4D memsets (one per dh) keep the APs within dim limits
            nc.vector.memset(vwx[:, :, :, 0:64], 1.0)
            nc.vector.memset(vwx[:, :, :, 128:192], 1.0)

            # A2A staging: cin rows [(2ch+h)*128 + dh*64 + d] = ctx^T rows,
            # chunk j of 128 rows goes to core j (= q block [256j, 256j+256))
            cin0 = dram.tile([1024, QO], BF16, name="cin0")
            cout0 = dram.tile([1024, QO], BF16, name="cout0")
            cin1 = dram.tile([1024, QO], BF16, name="cin1")
            cout1 = dram.tile([1024, QO], BF16, name="cout1")
            cins, couts = (cin0, cin1), (cout0, cout1)
            # full-size scratch warmup exchange absorbs the NRT
            # first-collective setup cost off the critical path
            ccw_in = dram.tile([1024, QO], BF16, name="ccw_in")
            ccw_out = dram.tile([1024, QO], BF16, name="ccw_out")

            # ---- startup DMAs, strict need-order -------------------------
            # first matmul needs kt chunk 0 + wk only: keep them unblocked
            def xchunk_dma(xtp, x_dram, cidx, name, qa, qb):
                t = xtp.tile([128, DKT, QW], BF16, name=name, tag="xt")
                h = DKT // 2
                qa.dma_start(out=t[:, 0:h, :], in_=x_dram[cidx][:, 0:h, :])
                qb.dma_start(out=t[:, h:DKT, :], in_=x_dram[cidx][:, h:DKT, :])
                return t

            with tc.tile_pool(name="xt", bufs=4) as xtp, \
                 tc.tile_pool(name="vts", bufs=2) as vts, \
                 tc.tile_pool(name="stg", bufs=4) as stp, \
                 tc.tile_pool(name="osb", bufs=2) as osb, \
                 tc.tile_pool(name="prp", bufs=40) as prp, \
                 tc.tile_pool(name="sps", bufs=2, space="PSUM") as sps, \
                 tc.tile_pool(name="vps", bufs=2, space="PSUM") as vps, \
                 tc.tile_pool(name="aps", bufs=2, space="PSUM") as aps:

                # x-chunk DMA emission must match consumption order exactly:
                # the 4-buffer ring makes chunk i's DMA wait on chunk i-4's
                # last reader, so out-of-order emission would deadlock the
                # in-order engines.
                kts, qts, vtss = {}, {}, {}
                kts[0] = xchunk_dma(xtp, kT, 0, "kt0", nc.sync, nc.scalar)
                nc.gpsimd.dma_start(out=wk_sb[:], in_=wk[:])
                nc.gpsimd.dma_start(out=bk_sb[:], in_=bk1[:])
                nc.gpsimd.dma_start(out=bq_sb[:], in_=bq1[:])
                nc.gpsimd.dma_start(out=bv_sb[:], in_=bv1[:])
                nc.gpsimd.dma_start(out=bo_sb[:], in_=bo1[:])
                nc.gpsimd.dma_start(out=id_sb[:], in_=ident[:])
                for c in range(1, 4):
                    kts[c] = xchunk_dma(xtp, kT, c, f"kt{c}", nc.sync,
                                        nc.scalar)
                nc.gpsimd.dma_start(out=wq_sb[:], in_=wq[:])
                qts[0] = xchunk_dma(xtp, qT, 0, "qt0", nc.sync, nc.scalar)
                nc.gpsimd.dma_start(out=wv_sb[:], in_=wv[:])
                for c in range(4):
                    vtss[c] = xchunk_dma(xtp, vT, c, f"vt{c}", nc.sync,
                                         nc.scalar)
                # collective warmup fires now; CC setup cost retires long
                # before the first real A2A
                nc.gpsimd.collective_compute(
                    "AllToAll", mybir.AluOpType.bypass,
                    replica_groups=[list(range(N_CORES))],
                    ins=[ccw_in[:].opt()], outs=[ccw_out[:].opt()])
                qts[1] = xchunk_dma(xtp, qT, 1, "qt1", nc.sync, nc.scalar)
                qts[2] = xchunk_dma(xtp, qT, 2, "qt2", nc.sync, nc.scalar)
                vtss[4] = xchunk_dma(xtp, vT, 4, "vt4", nc.sync, nc.scalar)
                qts[3] = xchunk_dma(xtp, qT, 3, "qt3", nc.sync, nc.scalar)
                vtss[5] = xchunk_dma(xtp, vT, 5, "vt5", nc.sync, nc.scalar)
                vtss[6] = xchunk_dma(xtp, vT, 6, "vt6", nc.sync, nc.scalar)
                vtss[7] = xchunk_dma(xtp, vT, 7, "vt7", nc.sync, nc.scalar)
                for c in range(4, 8):
                    kts[c] = xchunk_dma(xtp, kT, c, f"kt{c}", nc.sync,
                                        nc.scalar)
                nc.gpsimd.dma_start(out=wo_sb[:], in_=wo1[:])
                for c in range(4, 8):
                    qts[c] = xchunk_dma(xtp, qT, c, f"qt{c}", nc.sync,
                                        nc.scalar)

                # ---- projection emitters --------------------------------
                def emit_qk(w_sb, b_sb, dstT, xt, b, ch):
                    """One chunk of qw^T/kw^T: [128 dout, 512 q] += bias."""
                    ps = aps.tile([128, QW], F32, name="ps", tag="ps")
                    for kk in range(DKT):
                        nc.tensor.matmul(ps[:], w_sb[:, kk, :], xt[:, kk, :],
                                         start=(kk == 0), stop=(kk == DKT - 1))
                    nc.vector.tensor_scalar_add(
                        dstT[:, b, ch * QW:(ch + 1) * QW], ps[:],
                        b_sb[:, 0:1])

                def emit_vw(xt, b, ch):
                    """One chunk of vw: project transposed (N=512), add bias
                    per-partition, then PE-transpose 128x128 blocks into the
                    natural [kpos, ch] slots of vwx."""
                    ps = aps.tile([128, QW], F32, name="ps", tag="ps")
                    for kk in range(DKT):
                        nc.tensor.matmul(ps[:], wv_sb[:, kk, :], xt[:, kk, :],
                                         start=(kk == 0), stop=(kk == DKT - 1))
                    vt_sb = vts.tile([128, QW], BF16, name="vt_sb", tag="vt")
                    nc.vector.tensor_scalar_add(vt_sb[:], ps[:], bv_sb[:, 0:1])
                    for s in range(4):
                        kt = ch * 4 + s
                        tp = vps.tile([128, QW], F32, name="pv", tag="pv")
                        tpb = tp[:].bitcast(BF16)[:, 0:128]
                        nc.tensor.transpose(
                            tpb, vt_sb[:, s * 128:(s + 1) * 128], id_sb[:])
                        dst = vwx[:, b, kt, :].rearrange(
                            "p (d c) -> p d c", d=2)
                        nc.vector.tensor_copy(
                            dst[:, :, 64:128],
                            tpb.rearrange("p (d c) -> p d c", d=2))

                # ---- attention emitters ---------------------------------
                def emit_scores(b, ch, kt_range):
                    prs = []
                    for kt in kt_range:
                        sq = sps.tile([128, 2, QW], F32, name="sq", tag="sq")
                        for dh in range(2):
                            nc.tensor.matmul(
                                sq[:, dh, :],
                                kwT[dh * 64:(dh + 1) * 64, b,
                                    kt * 128:(kt + 1) * 128],
                                qwT[dh * 64:(dh + 1) * 64, b,
                                    ch * QW:(ch + 1) * QW],
                                start=True, stop=True)
                        pr = prp.tile([128, 2, QW], BF16, name="pr", tag="pr")
                        nc.scalar.activation(pr[:], sq[:], AF.Exp, scale=0.125)
                        prs.append(pr)
                    return prs

                def emit_pvs(b, ch, prs, dhs=(0, 1)):
                    for dh in dhs:
                        pv = vps.tile([128, QW], F32, name="pv", tag="pv")
                        for kt in range(KT):
                            nc.tensor.matmul(
                                pv[:],
                                vwx[:, b, kt, dh * 128:(dh + 1) * 128],
                                prs[kt][:, dh, :],
                                start=(kt == 0), stop=(kt == KT - 1))
                        # sums land at PSUM rows 0:64 (ones first in lhsT)
                        rec = stp.tile([64, QW], F32, name="rec", tag="rec")
                        nc.vector.reciprocal_approx_fast(rec[:], pv[0:64, :])
                        stg = stp.tile([64, QW], BF16, name="stg", tag="stg")
                        nc.vector.tensor_tensor(stg[:], pv[64:128, :], rec[:],
                                                MUL)
                        for h in range(2):
                            r0 = (2 * ch + h) * 128 + dh * 64
                            nc.sync.dma_start(
                                out=cins[b][r0:r0 + 64, :],
                                in_=stg[:, h * QO:(h + 1) * QO])

                def emit_a2a(b):
                    nc.gpsimd.collective_compute(
                        "AllToAll", mybir.AluOpType.bypass,
                        replica_groups=[list(range(N_CORES))],
                        ins=[cins[b][:].opt()],
                        outs=[couts[b][:].opt()])

                def emit_gth(gth, cout, qa, qb):
                    src = cout.rearrange("(k p) n -> p k n", p=128)
                    qa.dma_start(out=gth[:, 0:DKT // 2, :],
                                 in_=src[:, 0:DKT // 2, :])
                    qb.dma_start(out=gth[:, DKT // 2:DKT, :],
                                 in_=src[:, DKT // 2:DKT, :])

                def emit_oproj(b, gth, mb):
                    """Output projection for q-block mb of batch b."""
                    osb_t = osb.tile([128, D], F32, name="osb_t", tag="osb")
                    for nch in range(2):
                        ps = aps.tile([128, QW], F32, name="ps", tag="ps")
                        nc.tensor.matmul(ps[:], onesb[:],
                                         bo_sb[:, nch * QW:(nch + 1) * QW],
                                         start=True, stop=False)
                        for kk in range(DKT):
                            nc.tensor.matmul(
                                ps[:], gth[:, kk, mb * 128:(mb + 1) * 128],
                                wo_sb[:, kk, nch * QW:(nch + 1) * QW],
                                start=False, stop=(kk == DKT - 1))
                        nc.vector.tensor_copy(
                            osb_t[:, nch * QW:(nch + 1) * QW], ps[:])
                    nc.sync.dma_start(out=out[b, mb * 128:(mb + 1) * 128, :],
                                      in_=osb_t[:])

                # ---- phase 1a: batch-0 projections ----------------------
                # everything the first PV reads (all of vwx batch 0) must be
                # emitted before the attention loop: the tensor engine runs
                # in program order, so a PV ahead of its vw transposes in the
                # stream would deadlock.
                for ch in range(4):
                    emit_qk(wk_sb, bk_sb, kwT, kts[ch], 0, ch)
                emit_qk(wq_sb, bq_sb, qwT, qts[0], 0, 0)
                for ch in range(4):
                    emit_vw(vtss[ch], 0, ch)

                # ---- attention with aux work threaded through -----------
                # aux[(b, ch)] emitted after that slot's dh0 PV; everything
                # a later slot's scores/PVs read is emitted at least one
                # slot ahead of its first use.
                aux = {
                    (0, 0): [lambda: emit_qk(wq_sb, bq_sb, qwT, qts[1], 0, 1)],
                    (0, 1): [lambda: emit_qk(wq_sb, bq_sb, qwT, qts[2], 0, 2),
                             lambda: emit_vw(vtss[4], 1, 0)],
                    (0, 2): [lambda: emit_qk(wq_sb, bq_sb, qwT, qts[3], 0, 3),
                             lambda: emit_vw(vtss[5], 1, 1),
                             lambda: emit_vw(vtss[6], 1, 2)],
                    (0, 3): [lambda: emit_vw(vtss[7], 1, 3),
                             lambda: emit_qk(wk_sb, bk_sb, kwT, kts[4], 1, 0),
                             lambda: emit_qk(wk_sb, bk_sb, kwT, kts[5], 1, 1),
                             lambda: emit_qk(wk_sb, bk_sb, kwT, kts[6], 1, 2),
                             lambda: emit_qk(wk_sb, bk_sb, kwT, kts[7], 1, 3),
                             lambda: emit_qk(wq_sb, bq_sb, qwT, qts[4], 1, 0)],
                    (1, 0): [lambda: emit_qk(wq_sb, bq_sb, qwT, qts[5], 1, 1),
                             lambda: emit_gth(gth0, cout0, nc.sync,
                                              nc.gpsimd)],
                    (1, 1): [lambda: emit_qk(wq_sb, bq_sb, qwT, qts[6], 1, 2),
                             lambda: emit_oproj(0, gth0, 0)],
                    (1, 2): [lambda: emit_qk(wq_sb, bq_sb, qwT, qts[7], 1, 3),
                             lambda: emit_oproj(0, gth0, 1)],
                    (1, 3): [],
                }
                pend = None
                for b in range(2):
                    for ch in range(NCH):
                        prs = emit_scores(b, ch, range(KT // 2))
                        if pend is not None:
                            emit_pvs(*pend, dhs=(1,))
                            if pend[0] == 0 and pend[1] == NCH - 1:
                                emit_a2a(0)
                        prs += emit_scores(b, ch, range(KT // 2, KT))
                        emit_pvs(b, ch, prs, dhs=(0,))
                        for fn in aux[(b, ch)]:
                            fn()
                        pend = (b, ch, prs)
                emit_pvs(*pend, dhs=(1,))
                emit_a2a(1)

                if taps:
                    nc.sync.dma_start(out=taps["tqwT"][:], in_=qwT[:])
                    nc.sync.dma_start(out=taps["tkwT"][:], in_=kwT[:])
                    nc.sync.dma_start(out=taps["tvwx"][:], in_=vwx[:])

                # ---- tail: batch-1 out-projection after the A2A ---------
                # keep the PE clock warm across the collective wait
                warm = aps.tile([128, QW], F32, name="ps", tag="ps")
                for i in range(24):
                    nc.tensor.matmul(warm[:], onesb[:], bo_sb[:, 0:QW],
                                     start=(i == 0), stop=(i == 23))
                emit_gth(gth1, cout1, nc.sync, nc.gpsimd)
                emit_oproj(1, gth1, 0)
                emit_oproj(1, gth1, 1)

    nc.compile()
    return nc


def _get_nc():
    global _CACHED_NC
    if _CACHED_NC is None:
        _CACHED_NC = _build()
    return _CACHED_NC


def kernel(q, k, v, Wq, bq, Wk, bk, Wv, bv, Wo, bo, _return_results=False):
    q, k, v = (np.asarray(x, np.float32) for x in (q, k, v))
    Wq, bq, Wk, bk, Wv, bv, Wo, bo = (
        np.asarray(x, np.float32) for x in (Wq, bq, Wk, bk, Wv, bv, Wo, bo))

    nc = _get_nc()

    def arrange(x):  # [B, S, D] -> [2*NCH, 128, DKT, QW], same for all cores
        per_b = [np.ascontiguousarray(
            x[b].T.reshape(DKT, 128, NCH, QW).transpose(2, 1, 0, 3))
            for b in range(B)]
        return np.concatenate(per_b, axis=0).astype(ml_dtypes.bfloat16)

    qA, kA, vA = arrange(q), arrange(k), arrange(v)

    # Wo rows permuted to gathered-channel order (same for all cores)
    perm = np.empty(D, np.int64)
    for j in range(8):
        for dh in range(2):
            for d0 in range(64):
                perm[j * 128 + dh * 64 + d0] = (2 * j + dh) * 64 + d0
    woA = np.ascontiguousarray(
        Wo[perm].reshape(DKT, 128, D).transpose(1, 0, 2)).astype(
        ml_dtypes.bfloat16)
    identA = np.eye(128, dtype=np.float32).astype(ml_dtypes.bfloat16)
    boA = bo.reshape(1, D).astype(ml_dtypes.bfloat16)

    def warrange(w):  # [D, n] -> [128, DKT, n]
        n = w.shape[1]
        return np.ascontiguousarray(
            w.reshape(DKT, 128, n).transpose(1, 0, 2)).astype(
            ml_dtypes.bfloat16)

    in_maps = []
    for c in range(N_CORES):
        cols = slice(c * DPC, (c + 1) * DPC)
        in_maps.append({
            "qT": qA, "kT": kA, "vT": vA,
            "wq": warrange(Wq[:, cols]),
            "wk": warrange(Wk[:, cols]),
            "wv": warrange(Wv[:, cols]),
            "bq1": np.ascontiguousarray(bq[cols].reshape(128, 1)),
            "bk1": np.ascontiguousarray(bk[cols].reshape(128, 1)),
            "bv1": np.ascontiguousarray(bv[cols].reshape(128, 1)),
            "wo1": woA, "bo1": boA, "ident": identA,
        })

    res = run_bass_kernel_spmd(nc, in_maps, core_ids=list(range(N_CORES)))

    full = np.empty((B, S, D), np.float32)
    for c in range(N_CORES):
        o = res.results[c]["out"]
        for b in range(B):
            full[b, c * QO:(c + 1) * QO] = o[b]
    if _return_results:
        return full, res
    return full
